# revision 1
# baseline (speedup 1.0000x reference)
"""NT-Xent contrastive loss (SimCLR-style) on 8 Trainium2 NeuronCores.

Problem: z1, z2 [4096, 256] fp32 -> scalar loss.
  zn = l2norm(z), z = concat(z1, z2) -> [8192, 256]
  sim = zn @ zn.T / 0.07              -> [8192, 8192]
  loss = -mean_i log_softmax(sim)[i, partner(i)],  partner(i) = (i + 4096) % 8192

Strategy (symmetric): exp(sim) is symmetric, so each unordered tile pair
{a, b} of the 64x64 grid of 128x128 blocks is computed ONCE. The core
owning row-tile a computes blocks (a, a+o mod 64) for o = 0..32 (the
o=32 pair is computed by both owners: 3% redundancy that keeps the
program SPMD-identical). Row sums of each exp block accumulate directly
(DVE reduce); the transpose credit for o = 1..31 comes from COLUMN sums
of the same exp block, computed on the otherwise-idle PE as ones^T @ E
matmuls accumulating in PSUM. Host adds the per-core partial sums.

Per-core input is the normalized z^T rolled so its own 1024 rows sit at
columns [0, 1024): every core runs one identical program, and the o-arcs
become contiguous column ranges [0, 5120) -- only 62.5% of z is even
loaded. Matmuls run in fp8e4m3 (values pre-scaled x16) with DoubleRow
packing K=256 into one pass; exp runs on the scalar engine PSUM->SBUF in
bf16. Tolerance is rel 2e-2; fp8 error lands ~1e-3.

PSUM discipline: an accumulation group conflicts with any other group
opened in the same bank while it is live (start zero-marks the whole
2KB bank row), so long-lived accumulators get exclusive banks. Banks
0-3: gram double-buffer ([128,1024] x 2). Banks 4-7: col-sum chunks
cc=2..5 (target tiles 8..23), held open across the whole q loop.
Chunks cc=6,7 and the 7 inter-core edge tiles (t=32..38) run at the
tail through the drained banks; the 7 intra-core edge tiles (t=1..7)
run mid-loop as brief transient groups inside a gram slot.

exp SBUF layout per q: [o32 | o0 | o1 | ... | o31], so the o=32 block
rides in gram block 0 and every ACTIVATE is a full [128, 1024].
"""

import numpy as np

import concourse.bass as bass
import concourse.tile as tile
from concourse import bacc, mybir
from concourse.bass_utils import run_bass_kernel_spmd

B, D = 4096, 256
N = 2 * B            # 8192 embeddings
NCORES = 8
NT = N // 128        # 64 tiles of 128 embeddings
Q = 8                # row tiles per core
ARC = 33             # column tiles per row tile (o = 0..32)
COLS = ARC * 128     # 4224
ZCOLS = (Q - 1 + 32 + 1) * 128   # 5120 rolled columns needed per core
TEMP = 0.07
ZSCALE = 16.0        # pre-scale before fp8 cast (keeps values in e4m3 normal range)
EXP_SCALE = 1.0 / (ZSCALE * ZSCALE * TEMP)

USE_FP8 = True       # fp8e4m3 + DoubleRow; False -> bf16 two-pass contraction

F32 = mybir.dt.float32
BF16 = mybir.dt.bfloat16
FP8 = mybir.dt.float8e4

EDGE_TILES = list(range(1, 8)) + list(range(32, 39))


PROBE = "full"   # hw_probe.py bisect knob: full | nocolsum | noreduce | min


def build_nc() -> bass.Bass:
    probe = PROBE
    do_colsum = probe in ("full", "noreduce", "nottr", "norowred")
    do_ttr = probe in ("full", "nocolsum", "norowred")
    do_rowred = probe in ("full", "nocolsum", "nottr")
    do_dve = do_ttr or do_rowred
    zdt = FP8 if USE_FP8 else BF16
    nc = bacc.Bacc("TRN2", target_bir_lowering=False, debug=False, num_devices=NCORES)
    zdr = nc.declare_dram_parameter("zdr", [128, 2, ZCOLS], zdt, isOutput=False)
    idt_d = nc.declare_dram_parameter("idt", [128, 128], F32, isOutput=False)
    ones_d = nc.declare_dram_parameter("ones", [128, 32], BF16, isOutput=False)
    rowsum_d = nc.declare_dram_parameter("rowsum", [128, Q], F32, isOutput=True)
    tgt_d = nc.declare_dram_parameter("tgt", [128, Q], F32, isOutput=True)
    mid_d = nc.declare_dram_parameter("mid", [1, 6 * 512], F32, isOutput=True)
    edge_d = nc.declare_dram_parameter("edge", [1, 14 * 128], F32, isOutput=True)

    with tile.TileContext(nc) as tc:
        with (
            tc.tile_pool(name="zp", bufs=1) as zp,
            tc.tile_pool(name="const", bufs=1) as constp,
            tc.tile_pool(name="expp", bufs=8) as expp,
            tc.tile_pool(name="dgp", bufs=2) as dgp,
            tc.tile_pool(name="stats", bufs=1) as statsp,
            tc.tile_pool(name="gram", bufs=2, space="PSUM") as gramp,
            tc.tile_pool(name="cacc", bufs=1, space="PSUM") as caccp,
        ):
            z = zp.tile([128, 2, ZCOLS], zdt)
            for i in range(4):
                w = ZCOLS // 4
                nc.sync.dma_start(
                    z[:, :, i * w:(i + 1) * w], zdr[:, :, i * w:(i + 1) * w]
                )
            idt = constp.tile([128, 128], F32)
            nc.sync.dma_start(idt[:], idt_d[:])
            ones = constp.tile([128, 32], BF16)
            nc.sync.dma_start(ones[:], ones_d[:])

            rowsum = statsp.tile([128, Q], F32)
            tgtt = statsp.tile([128, Q], F32)
            edgestage = statsp.tile([32, 14 * 128], F32)
            midstage = statsp.tile([32, 6 * 512], F32)
            zeros = statsp.tile([32, 512], F32)
            nc.any.memset(zeros[:], 0.0)
            # long-lived col-sum accumulator banks (chunks cc=2..5; banks
            # reused at the tail for cc=6,7 and the t>=32 edge tiles)
            macc = [
                caccp.tile([128, 512], F32, tag=f"M{i}", name=f"macc{i}")
                for i in range(4)
            ]

            def gram_mm(out_ap, qs, c0, c1):
                """out = z[:, qs:qs+128].T @ z[:, c0:c1] (scaled x256)."""
                if USE_FP8:
                    nc.tensor.matmul(
                        out_ap,
                        lhsT=z[:, :, qs:qs + 128],
                        rhs=z[:, :, c0:c1],
                        start=True,
                        stop=True,
                        perf_mode=mybir.MatmulPerfMode.DoubleRow,
                    )
                else:
                    for ko in range(2):
                        nc.tensor.matmul(
                            out_ap,
                            lhsT=z[:, ko, qs:qs + 128],
                            rhs=z[:, ko, c0:c1],
                            start=(ko == 0),
                            stop=(ko == 1),
                        )

            def dve_copy(dst, src):
                # PSUM -> SBUF drain; only one non-scalar input may be PSUM
                w = src.shape[-1]
                nc.vector.scalar_tensor_tensor(
                    out=dst,
                    in0=src,
                    scalar=0.0,
                    in1=zeros[:, 0:w],
                    op0=mybir.AluOpType.bypass,
                    op1=mybir.AluOpType.add,
                )

            ets = []

            def emit_gram(q):
                qs = 128 * q
                et = expp.tile([128, COLS], BF16, tag="et")
                ets.append(et)
                # block 0: [o32 | o0..o6.75] -> et[:, 0:1024]
                ps = gramp.tile([128, 1024], F32, tag="g")
                gram_mm(ps[:, 0:128], qs, qs + 4096, qs + 4224)      # o=32
                gram_mm(ps[:, 128:512], qs, qs, qs + 384)
                gram_mm(ps[:, 512:1024], qs, qs + 384, qs + 896)
                if do_ttr:
                    # target-pair diag of the o=32 block: mask with identity,
                    # then row-reduce (TensorTensorReduce crashes the device
                    # in this config, so use the two-instruction form)
                    dg = dgp.tile([128, 128], F32, tag="dg")
                    nc.vector.tensor_mul(dg[:], ps[:, 0:128], idt[:])
                    nc.vector.tensor_reduce(
                        out=tgtt[:, q:q + 1],
                        in_=dg[:],
                        axis=mybir.AxisListType.X,
                        op=mybir.AluOpType.add,
                    )
                nc.scalar.activation(
                    et[:, 0:1024], ps[:],
                    mybir.ActivationFunctionType.Exp, scale=EXP_SCALE,
                )
                for b in range(1, 4):
                    ps = gramp.tile([128, 1024], F32, tag="g")
                    base = qs + 896 + 1024 * (b - 1)
                    gram_mm(ps[:, 0:512], qs, base, base + 512)
                    gram_mm(ps[:, 512:1024], qs, base + 512, base + 1024)
                    nc.scalar.activation(
                        et[:, 1024 * b:1024 * (b + 1)], ps[:],
                        mybir.ActivationFunctionType.Exp, scale=EXP_SCALE,
                    )
                # arc tail x in [3968, 4096) (tile o=31)
                ps = gramp.tile([128, 1024], F32, tag="g")
                gram_mm(ps[:, 0:128], qs, qs + 3968, qs + 4096)
                nc.scalar.activation(
                    et[:, 4096:4224], ps[:, 0:128],
                    mybir.ActivationFunctionType.Exp, scale=EXP_SCALE,
                )
                if do_rowred:
                    nc.vector.tensor_reduce(
                        out=rowsum[:, q:q + 1],
                        in_=et[:],
                        axis=mybir.AxisListType.X,
                        op=mybir.AluOpType.add,
                    )

            # et offset of arc column x (x = rolled col - qs), o>=0 tiles
            # shifted +128 by the leading o32 block: et_off = x + 128
            def colsum(granule, tq, x0, w, start, stop):
                nc.tensor.matmul(
                    granule,
                    lhsT=ones[:],
                    rhs=ets[tq][:, x0 + 128:x0 + 128 + w],
                    start=start,
                    stop=stop,
                )

            def emit_mid_colsums(q):
                """Chunks cc=2..5 (target tiles 8..23): every q contributes;
                PE accumulates across q in 4 exclusive PSUM banks."""
                for cc in range(2, 6):
                    colsum(
                        macc[cc - 2][0:32, 0:512], q, 512 * cc - 128 * q, 512,
                        start=(q == 0), stop=(q == 7),
                    )

            def emit_edge(t, e, granule):
                """All contributions for edge tile t, back-to-back (brief
                transient group), then drained to SBUF staging."""
                lo = max(0, t - 31)
                hi = min(7, t - 1)
                for q2 in range(lo, hi + 1):
                    colsum(
                        granule, q2, 128 * (t - q2), 128,
                        start=(q2 == lo), stop=(q2 == hi),
                    )
                dve_copy(edgestage[:, 128 * e:128 * (e + 1)], granule)

            for q in range(Q):
                emit_gram(q)
                if q >= 1 and do_colsum:
                    emit_mid_colsums(q - 1)
                    # intra-core edge tile t=q needs ets[0..q-1] only
                    g = gramp.tile([128, 1024], F32, tag="g")
                    emit_edge(q, q - 1, g[0:32, 0:128])
            if do_colsum:
                emit_mid_colsums(Q - 1)

                # tail: drain cc=2..5, then run cc=6,7 + edges t=32..38
                # through the freed banks
                for cc in range(2, 6):
                    dve_copy(
                        midstage[:, 512 * (cc - 2):512 * (cc - 1)],
                        macc[cc - 2][0:32, 0:512],
                    )
                for cc in (6, 7):
                    gran = macc[cc - 6][0:32, 0:512]
                    for q2 in range(Q):
                        colsum(
                            gran, q2, 512 * cc - 128 * q2, 512,
                            start=(q2 == 0), stop=(q2 == 7),
                        )
                    dve_copy(midstage[:, 512 * (cc - 2):512 * (cc - 1)], gran)
                for e, t in enumerate(EDGE_TILES):
                    if t < 32:
                        continue   # done mid-loop
                    gran = macc[2 + (e % 2)][0:32, 0:128]
                    emit_edge(t, e, gran)

                nc.sync.dma_start(edge_d[:], edgestage[0:1, :])
                nc.sync.dma_start(mid_d[:], midstage[0:1, :])
            if not do_rowred:
                # probe mode: keep outputs written so the build passes
                nc.scalar.copy(rowsum[:], ets[0][:, 0:Q])
            if not do_ttr:
                nc.scalar.copy(tgtt[:], ets[0][:, 0:Q])
            nc.sync.dma_start(rowsum_d[:], rowsum[:])
            nc.sync.dma_start(tgt_d[:], tgtt[:])

    nc.compile()
    return nc


def make_in_maps(z1: np.ndarray, z2: np.ndarray) -> list[dict]:
    z = np.concatenate([np.asarray(z1), np.asarray(z2)], axis=0).astype(np.float64)
    zn = z / np.maximum(np.linalg.norm(z, axis=1, keepdims=True), 1e-12)
    ztn = np.ascontiguousarray(zn.T * ZSCALE)  # [256, 8192]
    zdt = mybir.dt.np(FP8 if USE_FP8 else BF16)
    ident = np.eye(128, dtype=np.float32)
    onesm = np.ones((128, 32), dtype=mybir.dt.np(BF16))
    in_maps = []
    for c in range(NCORES):
        rolled = np.roll(ztn, -1024 * c, axis=1)[:, :ZCOLS]
        # DoubleRow layout: [partition p, ko, x] = row (128*ko + p)
        zdr = np.ascontiguousarray(
            rolled.reshape(2, 128, ZCOLS).transpose(1, 0, 2)
        ).astype(zdt)
        in_maps.append({"zdr": zdr, "idt": ident, "ones": onesm})
    return in_maps


def assemble(results: list[dict]) -> np.float32:
    S = np.zeros(N, dtype=np.float64)
    tgt_all = np.zeros(N, dtype=np.float64)
    for c in range(NCORES):
        r = results[c]
        rowsum = r["rowsum"].astype(np.float64)
        tgt = r["tgt"].astype(np.float64)
        mid = r["mid"].astype(np.float64).reshape(6, 512)
        edge = r["edge"].astype(np.float64).reshape(14, 128)
        base = 1024 * c
        for q in range(Q):
            S[base + 128 * q: base + 128 * (q + 1)] += rowsum[:, q]
            tgt_all[base + 128 * q: base + 128 * (q + 1)] = tgt[:, q]
        for cc in range(2, 8):
            gidx = (512 * cc + np.arange(512) + base) % N
            S[gidx] += mid[cc - 2]
        for e, t in enumerate(EDGE_TILES):
            gidx = (128 * t + np.arange(128) + base) % N
            S[gidx] += edge[e]
    loss = np.mean(np.log(S) - tgt_all / (ZSCALE * ZSCALE * TEMP))
    return np.float32(loss)


_NC_CACHE: list = []


def kernel(z1: np.ndarray, z2: np.ndarray) -> np.ndarray:
    in_maps = make_in_maps(z1, z2)
    if not _NC_CACHE:
        _NC_CACHE.append(build_nc())
    nc = _NC_CACHE[0]
    res = run_bass_kernel_spmd(nc, in_maps, list(range(NCORES)))
    return assemble(res.results)


if __name__ == "__main__":
    rng = np.random.default_rng(0)
    z1 = rng.standard_normal((B, D), dtype=np.float32)
    z2 = rng.standard_normal((B, D), dtype=np.float32)
    print(kernel(z1, z2))



# revision 3
# speedup vs baseline: 1.1397x; 1.1397x over previous
"""NT-Xent contrastive loss (SimCLR-style) on 8 Trainium2 NeuronCores.

Problem: z1, z2 [4096, 256] fp32 -> scalar loss.
  zn = l2norm(z), z = concat(z1, z2) -> [8192, 256]
  sim = zn @ zn.T / 0.07              -> [8192, 8192]
  loss = -mean_i log_softmax(sim)[i, partner(i)],  partner(i) = (i + 4096) % 8192

Strategy (symmetric): exp(sim) is symmetric, so each unordered tile pair
{a, b} of the 64x64 grid of 128x128 blocks is computed ONCE. The core
owning row-tile a computes blocks (a, a+o mod 64) for o = 0..32 (the
o=32 pair is computed by both owners: 3% redundancy that keeps the
program SPMD-identical). Row sums come for free from the DVE via
tensor_scalar's accum_out (runs in the 4x perf mode on packed bf16, so
~1.1us per row tile instead of tensor_reduce's 4.5us); the transpose
credit for o = 1..31 comes from COLUMN sums of the same exp block,
computed on the PE as ones^T @ E matmuls accumulating in PSUM. Host
adds the per-core partial sums.

Per-core input is the normalized z^T rolled so its own 1024 rows sit at
columns [0, 1024): every core runs one identical program, and the o-arcs
become contiguous column ranges [0, 5120) -- only 62.5% of z is even
loaded. Matmuls run in fp8e4m3 (values pre-scaled x16) with DoubleRow
packing K=256 into one pass; exp runs on the scalar engine PSUM->SBUF in
bf16. Tolerance is rel 2e-2; fp8 error lands ~1e-3.

exp SBUF layout per q: [o0 | o1 | ... | o31 | o32] (4096 + 128 = 4224),
so gram blocks are clean 1024-wide slabs [qs+1024b, qs+1024(b+1)) that
align with the z DMA chunks, and the o=32 tail (which needs the last z
columns) is emitted LAST -- the first matmul only waits on the first
1024-column DMA chunk. Column-sum matmuls are interleaved between gram
blocks as PE filler so the tensor engine stays busy (and stays out of
the low p-states) while ACT drains the previous block.

PSUM discipline: an accumulation group conflicts with any other group
opened in the same bank while it is live, so long-lived accumulators
get exclusive banks. Banks 0-3: gram double-buffer ([128,1024] x 2).
Banks 4-7: col-sum chunks cc=2..5 (target tiles 8..23), held open
across the whole q loop. Chunks cc=6,7 and the 7 inter-core edge tiles
(t=32..38) run at the tail through the drained banks; the 7 intra-core
edge tiles (t=1..7) run mid-loop as brief transient groups inside a
gram slot (their DVE drain is quick now that the DVE is nearly idle).
"""

import numpy as np

import concourse.bass as bass
import concourse.tile as tile
from concourse import bacc, mybir
from concourse.bass_utils import run_bass_kernel_spmd

B, D = 4096, 256
N = 2 * B            # 8192 embeddings
NCORES = 8
NT = N // 128        # 64 tiles of 128 embeddings
Q = 8                # row tiles per core
ARC = 33             # column tiles per row tile (o = 0..32)
COLS = ARC * 128     # 4224
OCOLS = 32 * 128     # 4096: o0..o31 region; o32 tail at [4096, 4224)
ZCOLS = (Q - 1 + 32 + 1) * 128   # 5120 rolled columns needed per core
TEMP = 0.07
ZSCALE = 16.0        # pre-scale before fp8 cast (keeps values in e4m3 normal range)
EXP_SCALE = 1.0 / (ZSCALE * ZSCALE * TEMP)

F32 = mybir.dt.float32
BF16 = mybir.dt.bfloat16
FP8 = mybir.dt.float8e4

EDGE_TILES = list(range(1, 8)) + list(range(32, 39))

# Rowsum via DVE tensor_scalar accum_out (4x perf mode). False falls back
# to the slow tensor_reduce path (hw bisect knob).
ROWSUM_TS = True

# z DMA column chunks, issued in ascending order so early gram matmuls
# only wait on the first chunk.
ZCHUNKS = [(0, 1024), (1024, 2048), (2048, 3072), (3072, 4224), (4224, 5120)]


def build_nc() -> bass.Bass:
    nc = bacc.Bacc("TRN2", target_bir_lowering=False, debug=False, num_devices=NCORES)
    zdr = nc.declare_dram_parameter("zdr", [128, 2, ZCOLS], FP8, isOutput=False)
    idt_d = nc.declare_dram_parameter("idt", [128, 128], F32, isOutput=False)
    ones_d = nc.declare_dram_parameter("ones", [128, 32], BF16, isOutput=False)
    rowsum_d = nc.declare_dram_parameter("rowsum", [128, Q], F32, isOutput=True)
    tgt_d = nc.declare_dram_parameter("tgt", [128, Q], F32, isOutput=True)
    mid_d = nc.declare_dram_parameter("mid", [1, 6 * 512], F32, isOutput=True)
    edge_d = nc.declare_dram_parameter("edge", [1, 14 * 128], F32, isOutput=True)

    with tile.TileContext(nc) as tc:
        with (
            tc.tile_pool(name="zp", bufs=1) as zp,
            tc.tile_pool(name="const", bufs=1) as constp,
            tc.tile_pool(name="expp", bufs=8) as expp,
            tc.tile_pool(name="dgp", bufs=2) as dgp,
            tc.tile_pool(name="scrp", bufs=1) as scrp,
            tc.tile_pool(name="stats", bufs=1) as statsp,
            tc.tile_pool(name="gram", bufs=2, space="PSUM") as gramp,
            tc.tile_pool(name="cacc", bufs=1, space="PSUM") as caccp,
        ):
            # ACT exp-table preload on a zeroed tile, overlapping the z DMA
            zeros = statsp.tile([32, 512], F32)
            nc.any.memset(zeros[:], 0.0)
            warm = statsp.tile([32, 2], F32)
            nc.scalar.activation(
                warm[:, 0:1], zeros[:, 0:1], mybir.ActivationFunctionType.Exp
            )

            # small constants first so they don't queue behind 1.3MB of z
            idt = constp.tile([128, 128], F32)
            nc.sync.dma_start(idt[:], idt_d[:])
            ones = constp.tile([128, 32], BF16)
            nc.sync.dma_start(ones[:], ones_d[:])

            z = zp.tile([128, 2, ZCOLS], FP8)
            for c0, c1 in ZCHUNKS:
                nc.sync.dma_start(z[:, :, c0:c1], zdr[:, :, c0:c1])

            rowsum = statsp.tile([128, Q], F32)
            tgtt = statsp.tile([128, Q], F32)
            edgestage = statsp.tile([32, 14 * 128], F32)
            midstage = statsp.tile([32, 6 * 512], F32)
            scr = scrp.tile([128, COLS], BF16)
            # long-lived col-sum accumulator banks (chunks cc=2..5; banks
            # reused at the tail for cc=6,7 and the t>=32 edge tiles)
            macc = [
                caccp.tile([128, 512], F32, tag=f"M{i}", name=f"macc{i}")
                for i in range(4)
            ]

            def gram_mm(out_ap, qs, c0, c1):
                """out = z[:, qs:qs+128].T @ z[:, c0:c1] (scaled x256)."""
                nc.tensor.matmul(
                    out_ap,
                    lhsT=z[:, :, qs:qs + 128],
                    rhs=z[:, :, c0:c1],
                    start=True,
                    stop=True,
                    perf_mode=mybir.MatmulPerfMode.DoubleRow,
                )

            def dve_copy(dst, src):
                # PSUM -> SBUF drain; only one non-scalar input may be PSUM
                w = src.shape[-1]
                nc.vector.scalar_tensor_tensor(
                    out=dst,
                    in0=src,
                    scalar=0.0,
                    in1=zeros[:, 0:w],
                    op0=mybir.AluOpType.bypass,
                    op1=mybir.AluOpType.add,
                )

            ets = []

            # et offset of arc column x (x = rolled col - qs): et_off = x
            def colsum(granule, tq, x0, w, start, stop):
                nc.tensor.matmul(
                    granule,
                    lhsT=ones[:],
                    rhs=ets[tq][:, x0:x0 + w],
                    start=start,
                    stop=stop,
                )

            def mid_colsum(q2, cc):
                """Chunk cc (target tiles 4cc..4cc+3): PE accumulates across
                q2 in an exclusive PSUM bank."""
                colsum(
                    macc[cc - 2][0:32, 0:512], q2, 512 * cc - 128 * q2, 512,
                    start=(q2 == 0), stop=(q2 == Q - 1),
                )

            def emit_edge(t, e, granule):
                """All contributions for edge tile t, back-to-back (brief
                transient group), then drained to SBUF staging."""
                lo = max(0, t - 31)
                hi = min(7, t - 1)
                for q2 in range(lo, hi + 1):
                    colsum(
                        granule, q2, 128 * (t - q2), 128,
                        start=(q2 == lo), stop=(q2 == hi),
                    )
                dve_copy(edgestage[:, 128 * e:128 * (e + 1)], granule)

            for q in range(Q):
                qs = 128 * q
                et = expp.tile([128, COLS], BF16, tag="et")
                ets.append(et)
                # gram blocks B0..B3: 1024-wide slabs, 2x512 matmuls each,
                # exp'd as soon as written; col-sum filler interleaved so
                # the PE never idles waiting on ACT to free a PSUM slot
                pb = []
                for b in range(2):
                    p = gramp.tile([128, 1024], F32, tag="g")
                    base = qs + 1024 * b
                    gram_mm(p[:, 0:512], qs, base, base + 512)
                    gram_mm(p[:, 512:1024], qs, base + 512, base + 1024)
                    pb.append(p)
                nc.scalar.activation(
                    et[:, 0:1024], pb[0][:],
                    mybir.ActivationFunctionType.Exp, scale=EXP_SCALE,
                )
                if q >= 1:
                    mid_colsum(q - 1, 2)
                nc.scalar.activation(
                    et[:, 1024:2048], pb[1][:],
                    mybir.ActivationFunctionType.Exp, scale=EXP_SCALE,
                )
                for b in range(2, 4):
                    p = gramp.tile([128, 1024], F32, tag="g")
                    base = qs + 1024 * b
                    gram_mm(p[:, 0:512], qs, base, base + 512)
                    gram_mm(p[:, 512:1024], qs, base + 512, base + 1024)
                    if q >= 1:
                        mid_colsum(q - 1, b + 1)
                    nc.scalar.activation(
                        et[:, 1024 * b:1024 * (b + 1)], p[:],
                        mybir.ActivationFunctionType.Exp, scale=EXP_SCALE,
                    )
                # B4: the o=32 tile [qs+4096, qs+4224); its PSUM holds the
                # target-pair diag, extracted on the DVE before exp
                p4 = gramp.tile([128, 1024], F32, tag="g")
                gram_mm(p4[:, 0:128], qs, qs + 4096, qs + 4224)
                if q >= 1:
                    mid_colsum(q - 1, 5)
                dg = dgp.tile([128, 128], F32, tag="dg")
                nc.vector.tensor_mul(dg[:], p4[:, 0:128], idt[:])
                nc.vector.tensor_reduce(
                    out=tgtt[:, q:q + 1],
                    in_=dg[:],
                    axis=mybir.AxisListType.X,
                    op=mybir.AluOpType.add,
                )
                nc.scalar.activation(
                    et[:, 4096:4224], p4[:, 0:128],
                    mybir.ActivationFunctionType.Exp, scale=EXP_SCALE,
                )
                # intra-core edge tile t=q needs ets[0..q-1] only
                if q >= 1:
                    g = gramp.tile([128, 1024], F32, tag="g")
                    emit_edge(q, q - 1, g[0:32, 0:128])
                # rowsum of the full 4224-wide row tile: DVE tensor_scalar
                # in the 4x perf mode, sum lands in accum_out
                if ROWSUM_TS:
                    nc.vector.tensor_scalar(
                        out=scr[:],
                        in0=et[:],
                        scalar1=1.0,
                        scalar2=None,
                        op0=mybir.AluOpType.mult,
                        op1=mybir.AluOpType.add,
                        accum_out=rowsum[:, q:q + 1],
                    )
                else:
                    nc.vector.tensor_reduce(
                        out=rowsum[:, q:q + 1],
                        in_=et[:],
                        axis=mybir.AxisListType.X,
                        op=mybir.AluOpType.add,
                    )

            for cc in range(2, 6):
                mid_colsum(Q - 1, cc)

            # tail: drain cc=2..5, then run cc=6,7 + edges t=32..38
            # through the freed banks
            for cc in range(2, 6):
                dve_copy(
                    midstage[:, 512 * (cc - 2):512 * (cc - 1)],
                    macc[cc - 2][0:32, 0:512],
                )
            for cc in (6, 7):
                gran = macc[cc - 6][0:32, 0:512]
                for q2 in range(Q):
                    colsum(
                        gran, q2, 512 * cc - 128 * q2, 512,
                        start=(q2 == 0), stop=(q2 == 7),
                    )
                dve_copy(midstage[:, 512 * (cc - 2):512 * (cc - 1)], gran)
            for e, t in enumerate(EDGE_TILES):
                if t < 32:
                    continue   # done mid-loop
                gran = macc[2 + (e % 2)][0:32, 0:128]
                emit_edge(t, e, gran)

            nc.sync.dma_start(edge_d[:], edgestage[0:1, :])
            nc.sync.dma_start(mid_d[:], midstage[0:1, :])
            nc.sync.dma_start(rowsum_d[:], rowsum[:])
            nc.sync.dma_start(tgt_d[:], tgtt[:])

    nc.compile()
    return nc


def make_in_maps(z1: np.ndarray, z2: np.ndarray) -> list[dict]:
    z = np.concatenate([np.asarray(z1), np.asarray(z2)], axis=0).astype(np.float64)
    zn = z / np.maximum(np.linalg.norm(z, axis=1, keepdims=True), 1e-12)
    ztn = np.ascontiguousarray(zn.T * ZSCALE)  # [256, 8192]
    zdt = mybir.dt.np(FP8)
    ident = np.eye(128, dtype=np.float32)
    onesm = np.ones((128, 32), dtype=mybir.dt.np(BF16))
    in_maps = []
    for c in range(NCORES):
        rolled = np.roll(ztn, -1024 * c, axis=1)[:, :ZCOLS]
        # DoubleRow layout: [partition p, ko, x] = row (128*ko + p)
        zdr = np.ascontiguousarray(
            rolled.reshape(2, 128, ZCOLS).transpose(1, 0, 2)
        ).astype(zdt)
        in_maps.append({"zdr": zdr, "idt": ident, "ones": onesm})
    return in_maps


def assemble(results: list[dict]) -> np.float32:
    S = np.zeros(N, dtype=np.float64)
    tgt_all = np.zeros(N, dtype=np.float64)
    for c in range(NCORES):
        r = results[c]
        rowsum = r["rowsum"].astype(np.float64)
        tgt = r["tgt"].astype(np.float64)
        mid = r["mid"].astype(np.float64).reshape(6, 512)
        edge = r["edge"].astype(np.float64).reshape(14, 128)
        base = 1024 * c
        for q in range(Q):
            S[base + 128 * q: base + 128 * (q + 1)] += rowsum[:, q]
            tgt_all[base + 128 * q: base + 128 * (q + 1)] = tgt[:, q]
        for cc in range(2, 8):
            gidx = (512 * cc + np.arange(512) + base) % N
            S[gidx] += mid[cc - 2]
        for e, t in enumerate(EDGE_TILES):
            gidx = (128 * t + np.arange(128) + base) % N
            S[gidx] += edge[e]
    loss = np.mean(np.log(S) - tgt_all / (ZSCALE * ZSCALE * TEMP))
    return np.float32(loss)


_NC_CACHE: list = []


def kernel(z1: np.ndarray, z2: np.ndarray) -> np.ndarray:
    in_maps = make_in_maps(z1, z2)
    if not _NC_CACHE:
        _NC_CACHE.append(build_nc())
    nc = _NC_CACHE[0]
    res = run_bass_kernel_spmd(nc, in_maps, list(range(NCORES)))
    return assemble(res.results)


if __name__ == "__main__":
    rng = np.random.default_rng(0)
    z1 = rng.standard_normal((B, D), dtype=np.float32)
    z2 = rng.standard_normal((B, D), dtype=np.float32)
    print(kernel(z1, z2))


# revision 11
# speedup vs baseline: 1.2982x; 1.1391x over previous
"""NT-Xent contrastive loss (SimCLR-style) on 8 Trainium2 NeuronCores.

Problem: z1, z2 [4096, 256] fp32 -> scalar loss.
  zn = l2norm(z), z = concat(z1, z2) -> [8192, 256]
  sim = zn @ zn.T / 0.07              -> [8192, 8192]
  loss = -mean_i log_softmax(sim)[i, partner(i)],  partner(i) = (i + 4096) % 8192

Strategy (symmetric): exp(sim) is symmetric, so each unordered tile pair
{a, b} of the 64x64 grid of 128x128 blocks is computed ONCE. The core
owning row-tile a computes blocks (a, a+o mod 64) for o = 0..32 (the
o=32 pair is computed by both owners: 3% redundancy that keeps the
program SPMD-identical). Row sums come for free from the DVE via
tensor_scalar's accum_out (runs in the 4x perf mode on packed bf16, so
~1.1us per row tile instead of tensor_reduce's 4.5us); the transpose
credit for o = 1..31 comes from COLUMN sums of the same exp block,
computed on the PE as ones^T @ E matmuls accumulating in PSUM. Host
adds the per-core partial sums.

Per-core input is the normalized z^T rolled so its own 1024 rows sit at
columns [0, 1024): every core runs one identical program, and the o-arcs
become contiguous column ranges [0, 5120) -- only 62.5% of z is even
loaded. Matmuls run in fp8e4m3 (values pre-scaled x16) with DoubleRow
packing K=256 into one pass; exp runs on the scalar engine PSUM->SBUF in
bf16. Tolerance is rel 2e-2; fp8 error lands ~1e-3.

exp SBUF layout per q: [o0 | o1 | ... | o31 | o32] (4096 + 128 = 4224),
so gram blocks are clean 1024-wide slabs [qs+1024b, qs+1024(b+1)) that
align with the z DMA chunks, and the o=32 tail (which needs the last z
columns) is emitted LAST -- the first matmul only waits on the first
1024-column DMA chunk. Column-sum matmuls are interleaved between gram
blocks as PE filler so the tensor engine stays busy (and stays out of
the low p-states) while ACT drains the previous block.

PSUM discipline: an accumulation group conflicts with any other group
opened in the same bank while it is live, so long-lived accumulators
get exclusive banks. Banks 0-3: gram double-buffer ([128,1024] x 2).
Banks 4-7: col-sum chunks cc=2..5 (target tiles 8..23), held open
across the whole q loop. Chunks cc=6,7 and the 7 inter-core edge tiles
(t=32..38) run at the tail through the drained banks; the 7 intra-core
edge tiles (t=1..7) run mid-loop as brief transient groups inside a
gram slot (their DVE drain is quick now that the DVE is nearly idle).
"""

import numpy as np

import concourse.bass as bass
import concourse.tile as tile
from concourse import bacc, mybir
from concourse.bass_utils import run_bass_kernel_spmd

B, D = 4096, 256
N = 2 * B            # 8192 embeddings
NCORES = 8
NT = N // 128        # 64 tiles of 128 embeddings
Q = 8                # row tiles per core
ARC = 33             # column tiles per row tile (o = 0..32)
COLS = ARC * 128     # 4224
OCOLS = 32 * 128     # 4096: o0..o31 region; o32 tail at [4096, 4224)
ZCOLS = (Q - 1 + 32 + 1) * 128   # 5120 rolled columns needed per core
TEMP = 0.07
ZSCALE = 16.0        # pre-scale before fp8 cast (keeps values in e4m3 normal range)
EXP_SCALE = 1.0 / (ZSCALE * ZSCALE * TEMP)

F32 = mybir.dt.float32
BF16 = mybir.dt.bfloat16
FP8 = mybir.dt.float8e4

EDGE_TILES = list(range(1, 8)) + list(range(32, 39))

# Rowsum strategy: the DVE reduce-with-accumulator paths all run at
# 1 el/cycle on hw (the 2x/4x packed modes don't apply to reductions), so
# a monolithic 4224-wide reduce costs 4.5us and serializes the pipeline.
# Split it instead: the B0 block's sum rides the ACT exp instruction via
# accum_out (~0.2-0.3us accumulator read), and B1, B2, B3+B4 get their
# own DVE tensor_reduce partials (~1.1us each, interleaved with drains).
# Host adds the 4 partials per row tile.
RS_PARTS = 4         # rowsum partials per q: [ACT B0, DVE B1, DVE B2, DVE B3+B4]

# z DMA column chunks, issued in ascending order so early gram matmuls
# only wait on the first chunk.
ZCHUNKS = [(0, 1024), (1024, 2048), (2048, 3072), (3072, 4224), (4224, 5120)]


def build_nc() -> bass.Bass:
    nc = bacc.Bacc("TRN2", target_bir_lowering=False, debug=False, num_devices=NCORES)
    zdr = nc.declare_dram_parameter("zdr", [128, 2, ZCOLS], FP8, isOutput=False)
    idt_d = nc.declare_dram_parameter("idt", [128, 128], F32, isOutput=False)
    ones_d = nc.declare_dram_parameter("ones", [128, 32], BF16, isOutput=False)
    rowsum_d = nc.declare_dram_parameter("rowsum", [128, RS_PARTS * Q], F32, isOutput=True)
    tgt_d = nc.declare_dram_parameter("tgt", [128, Q], F32, isOutput=True)
    mid_d = nc.declare_dram_parameter("mid", [1, 6 * 512], F32, isOutput=True)
    edge_d = nc.declare_dram_parameter("edge", [1, 14 * 128], F32, isOutput=True)

    with tile.TileContext(nc) as tc:
        with (
            tc.tile_pool(name="zp", bufs=1) as zp,
            tc.tile_pool(name="const", bufs=1) as constp,
            tc.tile_pool(name="expp", bufs=8) as expp,
            tc.tile_pool(name="dgp", bufs=2) as dgp,
            tc.tile_pool(name="stats", bufs=1) as statsp,
            tc.tile_pool(name="gram", bufs=2, space="PSUM") as gramp,
            tc.tile_pool(name="cacc", bufs=1, space="PSUM") as caccp,
        ):
            # ACT exp-table preload on a zeroed tile, overlapping the z DMA
            zeros = statsp.tile([32, 512], F32)
            nc.any.memset(zeros[:], 0.0)
            warm = statsp.tile([32, 2], F32)
            nc.scalar.activation(
                warm[:, 0:1], zeros[:, 0:1], mybir.ActivationFunctionType.Exp
            )

            # small constants first so they don't queue behind 1.3MB of z
            idt = constp.tile([128, 128], F32)
            nc.sync.dma_start(idt[:], idt_d[:])
            ones = constp.tile([128, 32], BF16)
            nc.sync.dma_start(ones[:], ones_d[:])

            z = zp.tile([128, 2, ZCOLS], FP8)
            for c0, c1 in ZCHUNKS:
                nc.sync.dma_start(z[:, :, c0:c1], zdr[:, :, c0:c1])

            rowsum = statsp.tile([128, RS_PARTS * Q], F32)
            tgtt = statsp.tile([128, Q], F32)
            edgestage = statsp.tile([32, 14 * 128], F32)
            midstage = statsp.tile([32, 6 * 512], F32)
            # long-lived col-sum accumulator banks (chunks cc=2..5; banks
            # reused at the tail for cc=6,7 and the t>=32 edge tiles)
            macc = [
                caccp.tile([128, 512], F32, tag=f"M{i}", name=f"macc{i}")
                for i in range(4)
            ]

            def gram_mm(out_ap, qs, c0, c1):
                """out = z[:, qs:qs+128].T @ z[:, c0:c1] (scaled x256)."""
                nc.tensor.matmul(
                    out_ap,
                    lhsT=z[:, :, qs:qs + 128],
                    rhs=z[:, :, c0:c1],
                    start=True,
                    stop=True,
                    perf_mode=mybir.MatmulPerfMode.DoubleRow,
                )

            def dve_copy(dst, src):
                # PSUM -> SBUF drain; only one non-scalar input may be PSUM
                w = src.shape[-1]
                nc.vector.scalar_tensor_tensor(
                    out=dst,
                    in0=src,
                    scalar=0.0,
                    in1=zeros[:, 0:w],
                    op0=mybir.AluOpType.bypass,
                    op1=mybir.AluOpType.add,
                )

            ets = []

            # et offset of arc column x (x = rolled col - qs): et_off = x
            def colsum(granule, tq, x0, w, start, stop):
                nc.tensor.matmul(
                    granule,
                    lhsT=ones[:],
                    rhs=ets[tq][:, x0:x0 + w],
                    start=start,
                    stop=stop,
                )

            def mid_colsum(q2, cc):
                """Chunk cc (target tiles 4cc..4cc+3): PE accumulates across
                q2 in an exclusive PSUM bank."""
                colsum(
                    macc[cc - 2][0:32, 0:512], q2, 512 * cc - 128 * q2, 512,
                    start=(q2 == 0), stop=(q2 == Q - 1),
                )

            def emit_edge(t, e, granule):
                """All contributions for edge tile t, back-to-back (brief
                transient group), then drained to SBUF staging."""
                lo = max(0, t - 31)
                hi = min(7, t - 1)
                for q2 in range(lo, hi + 1):
                    colsum(
                        granule, q2, 128 * (t - q2), 128,
                        start=(q2 == lo), stop=(q2 == hi),
                    )
                dve_copy(edgestage[:, 128 * e:128 * (e + 1)], granule)

            for q in range(Q):
                qs = 128 * q
                et = expp.tile([128, COLS], BF16, tag="et")
                ets.append(et)
                # gram blocks B0..B3: 1024-wide slabs, 2x512 matmuls each,
                # exp'd as soon as written; col-sum filler interleaved so
                # the PE never idles waiting on ACT to free a PSUM slot
                pb = []
                for b in range(2):
                    p = gramp.tile([128, 1024], F32, tag="g")
                    base = qs + 1024 * b
                    gram_mm(p[:, 0:512], qs, base, base + 512)
                    gram_mm(p[:, 512:1024], qs, base + 512, base + 1024)
                    pb.append(p)
                # B0's rowsum partial rides the exp via the ACT accumulator
                nc.scalar.activation(
                    et[:, 0:1024], pb[0][:],
                    mybir.ActivationFunctionType.Exp, scale=EXP_SCALE,
                    accum_out=rowsum[:, RS_PARTS * q:RS_PARTS * q + 1],
                )
                if q >= 1:
                    mid_colsum(q - 1, 2)
                nc.scalar.activation(
                    et[:, 1024:2048], pb[1][:],
                    mybir.ActivationFunctionType.Exp, scale=EXP_SCALE,
                )
                nc.vector.tensor_reduce(
                    out=rowsum[:, RS_PARTS * q + 1:RS_PARTS * q + 2],
                    in_=et[:, 1024:2048],
                    axis=mybir.AxisListType.X,
                    op=mybir.AluOpType.add,
                )
                for b in range(2, 4):
                    p = gramp.tile([128, 1024], F32, tag="g")
                    base = qs + 1024 * b
                    gram_mm(p[:, 0:512], qs, base, base + 512)
                    gram_mm(p[:, 512:1024], qs, base + 512, base + 1024)
                    if q >= 1:
                        mid_colsum(q - 1, b + 1)
                    nc.scalar.activation(
                        et[:, 1024 * b:1024 * (b + 1)], p[:],
                        mybir.ActivationFunctionType.Exp, scale=EXP_SCALE,
                    )
                nc.vector.tensor_reduce(
                    out=rowsum[:, RS_PARTS * q + 2:RS_PARTS * q + 3],
                    in_=et[:, 2048:3072],
                    axis=mybir.AxisListType.X,
                    op=mybir.AluOpType.add,
                )
                # B4: the o=32 tile [qs+4096, qs+4224); its PSUM holds the
                # target-pair diag, extracted on the DVE before exp
                p4 = gramp.tile([128, 1024], F32, tag="g")
                gram_mm(p4[:, 0:128], qs, qs + 4096, qs + 4224)
                if q >= 1:
                    mid_colsum(q - 1, 5)
                dg = dgp.tile([128, 128], F32, tag="dg")
                nc.vector.tensor_mul(dg[:], p4[:, 0:128], idt[:])
                nc.vector.tensor_reduce(
                    out=tgtt[:, q:q + 1],
                    in_=dg[:],
                    axis=mybir.AxisListType.X,
                    op=mybir.AluOpType.add,
                )
                nc.scalar.activation(
                    et[:, 4096:4224], p4[:, 0:128],
                    mybir.ActivationFunctionType.Exp, scale=EXP_SCALE,
                )
                # B3+B4 are contiguous in et: one DVE partial for [3072:4224)
                nc.vector.tensor_reduce(
                    out=rowsum[:, RS_PARTS * q + 3:RS_PARTS * q + 4],
                    in_=et[:, 3072:4224],
                    axis=mybir.AxisListType.X,
                    op=mybir.AluOpType.add,
                )
                # intra-core edge tile t=q needs ets[0..q-1] only
                if q >= 1:
                    g = gramp.tile([128, 1024], F32, tag="g")
                    emit_edge(q, q - 1, g[0:32, 0:128])

            for cc in range(2, 6):
                mid_colsum(Q - 1, cc)

            # tail: drain cc=2..5, then run cc=6,7 + edges t=32..38
            # through the freed banks
            for cc in range(2, 6):
                dve_copy(
                    midstage[:, 512 * (cc - 2):512 * (cc - 1)],
                    macc[cc - 2][0:32, 0:512],
                )
            for cc in (6, 7):
                gran = macc[cc - 6][0:32, 0:512]
                for q2 in range(Q):
                    colsum(
                        gran, q2, 512 * cc - 128 * q2, 512,
                        start=(q2 == 0), stop=(q2 == 7),
                    )
                dve_copy(midstage[:, 512 * (cc - 2):512 * (cc - 1)], gran)
            for e, t in enumerate(EDGE_TILES):
                if t < 32:
                    continue   # done mid-loop
                gran = macc[2 + (e % 2)][0:32, 0:128]
                emit_edge(t, e, gran)

            nc.sync.dma_start(edge_d[:], edgestage[0:1, :])
            nc.sync.dma_start(mid_d[:], midstage[0:1, :])
            nc.sync.dma_start(rowsum_d[:], rowsum[:])
            nc.sync.dma_start(tgt_d[:], tgtt[:])

    nc.compile()
    return nc


def make_in_maps(z1: np.ndarray, z2: np.ndarray) -> list[dict]:
    z = np.concatenate([np.asarray(z1), np.asarray(z2)], axis=0).astype(np.float64)
    zn = z / np.maximum(np.linalg.norm(z, axis=1, keepdims=True), 1e-12)
    ztn = np.ascontiguousarray(zn.T * ZSCALE)  # [256, 8192]
    zdt = mybir.dt.np(FP8)
    ident = np.eye(128, dtype=np.float32)
    onesm = np.ones((128, 32), dtype=mybir.dt.np(BF16))
    in_maps = []
    for c in range(NCORES):
        rolled = np.roll(ztn, -1024 * c, axis=1)[:, :ZCOLS]
        # DoubleRow layout: [partition p, ko, x] = row (128*ko + p)
        zdr = np.ascontiguousarray(
            rolled.reshape(2, 128, ZCOLS).transpose(1, 0, 2)
        ).astype(zdt)
        in_maps.append({"zdr": zdr, "idt": ident, "ones": onesm})
    return in_maps


def assemble(results: list[dict]) -> np.float32:
    S = np.zeros(N, dtype=np.float64)
    tgt_all = np.zeros(N, dtype=np.float64)
    for c in range(NCORES):
        r = results[c]
        rowsum = (
            r["rowsum"].astype(np.float64).reshape(128, Q, RS_PARTS).sum(axis=-1)
        )
        tgt = r["tgt"].astype(np.float64)
        mid = r["mid"].astype(np.float64).reshape(6, 512)
        edge = r["edge"].astype(np.float64).reshape(14, 128)
        base = 1024 * c
        for q in range(Q):
            S[base + 128 * q: base + 128 * (q + 1)] += rowsum[:, q]
            tgt_all[base + 128 * q: base + 128 * (q + 1)] = tgt[:, q]
        for cc in range(2, 8):
            gidx = (512 * cc + np.arange(512) + base) % N
            S[gidx] += mid[cc - 2]
        for e, t in enumerate(EDGE_TILES):
            gidx = (128 * t + np.arange(128) + base) % N
            S[gidx] += edge[e]
    loss = np.mean(np.log(S) - tgt_all / (ZSCALE * ZSCALE * TEMP))
    return np.float32(loss)


_NC_CACHE: list = []


def kernel(z1: np.ndarray, z2: np.ndarray) -> np.ndarray:
    in_maps = make_in_maps(z1, z2)
    if not _NC_CACHE:
        _NC_CACHE.append(build_nc())
    nc = _NC_CACHE[0]
    res = run_bass_kernel_spmd(nc, in_maps, list(range(NCORES)))
    return assemble(res.results)


if __name__ == "__main__":
    rng = np.random.default_rng(0)
    z1 = rng.standard_normal((B, D), dtype=np.float32)
    z2 = rng.standard_normal((B, D), dtype=np.float32)
    print(kernel(z1, z2))


# revision 17
# speedup vs baseline: 1.3407x; 1.0327x over previous
"""NT-Xent contrastive loss (SimCLR-style) on 8 Trainium2 NeuronCores.

Problem: z1, z2 [4096, 256] fp32 -> scalar loss.
  zn = l2norm(z), z = concat(z1, z2) -> [8192, 256]
  sim = zn @ zn.T / 0.07              -> [8192, 8192]
  loss = -mean_i log_softmax(sim)[i, partner(i)],  partner(i) = (i + 4096) % 8192

Strategy (symmetric): exp(sim) is symmetric, so each unordered tile pair
{a, b} of the 64x64 grid of 128x128 blocks is computed ONCE. The core
owning row-tile a computes blocks (a, a+o mod 64) for o = 0..32 (the
o=32 pair is computed by both owners: 3% redundancy that keeps the
program SPMD-identical). Row sums come for free from the DVE via
tensor_scalar's accum_out (runs in the 4x perf mode on packed bf16, so
~1.1us per row tile instead of tensor_reduce's 4.5us); the transpose
credit for o = 1..31 comes from COLUMN sums of the same exp block,
computed on the PE as ones^T @ E matmuls accumulating in PSUM. Host
adds the per-core partial sums.

Per-core input is the normalized z^T rolled so its own 1024 rows sit at
columns [0, 1024): every core runs one identical program, and the o-arcs
become contiguous column ranges [0, 5120) -- only 62.5% of z is even
loaded. Matmuls run in fp8e4m3 (values pre-scaled x16) with DoubleRow
packing K=256 into one pass; exp runs on the scalar engine PSUM->SBUF in
bf16. Tolerance is rel 2e-2; fp8 error lands ~1e-3.

exp SBUF layout per q: [o0 | o1 | ... | o31 | o32] (4096 + 128 = 4224),
so gram blocks are clean 1024-wide slabs [qs+1024b, qs+1024(b+1)) that
align with the z DMA chunks, and the o=32 tail (which needs the last z
columns) is emitted LAST -- the first matmul only waits on the first
1024-column DMA chunk. Column-sum matmuls are interleaved between gram
blocks as PE filler so the tensor engine stays busy (and stays out of
the low p-states) while ACT drains the previous block.

PSUM discipline: an accumulation group conflicts with any other group
opened in the same bank while it is live, so long-lived accumulators
get exclusive banks. Banks 0-3: gram double-buffer ([128,1024] x 2).
Banks 4-7: col-sum chunks cc=2..5 (target tiles 8..23), held open
across the whole q loop. Chunks cc=6,7 and the 7 inter-core edge tiles
(t=32..38) run at the tail through the drained banks; the 7 intra-core
edge tiles (t=1..7) run mid-loop as brief transient groups inside a
gram slot (their DVE drain is quick now that the DVE is nearly idle).
"""

import numpy as np

import concourse.bass as bass
import concourse.tile as tile
from concourse import bacc, mybir
from concourse.bass_utils import run_bass_kernel_spmd

B, D = 4096, 256
N = 2 * B            # 8192 embeddings
NCORES = 8
NT = N // 128        # 64 tiles of 128 embeddings
Q = 8                # row tiles per core
ARC = 33             # column tiles per row tile (o = 0..32)
COLS = ARC * 128     # 4224
OCOLS = 32 * 128     # 4096: o0..o31 region; o32 tail at [4096, 4224)
ZCOLS = (Q - 1 + 32 + 1) * 128   # 5120 rolled columns needed per core
TEMP = 0.07
ZSCALE = 16.0        # pre-scale before fp8 cast (keeps values in e4m3 normal range)
EXP_SCALE = 1.0 / (ZSCALE * ZSCALE * TEMP)

F32 = mybir.dt.float32
BF16 = mybir.dt.bfloat16
FP8 = mybir.dt.float8e4

EDGE_TILES = list(range(1, 8)) + list(range(32, 39))

# Rowsum strategy: the DVE reduce-with-accumulator paths all run at
# 1 el/cycle on hw (the 2x/4x packed modes don't apply to reductions), so
# a monolithic 4224-wide reduce costs 4.5us and serializes the pipeline.
# Split it instead: the B0 block's sum rides the ACT exp instruction via
# accum_out (~0.2-0.3us accumulator read), and B1, B2, B3+B4 get their
# own DVE tensor_reduce partials (~1.1us each, interleaved with drains).
# Host adds the 4 partials per row tile.
RS_PARTS = 4         # rowsum partials per q: [ACT B0, DVE B1, DVE B2, DVE B3+B4]

# z DMA column chunks, issued in ascending order so early gram matmuls
# only wait on the first chunk.
ZCHUNKS = [(0, 1024), (1024, 2048), (2048, 3072), (3072, 4224), (4224, 5120)]


def build_nc() -> bass.Bass:
    nc = bacc.Bacc("TRN2", target_bir_lowering=False, debug=False, num_devices=NCORES)
    zdr = nc.declare_dram_parameter("zdr", [128, 2, ZCOLS], FP8, isOutput=False)
    idt_d = nc.declare_dram_parameter("idt", [128, 128], F32, isOutput=False)
    ones_d = nc.declare_dram_parameter("ones", [128, 32], BF16, isOutput=False)
    rowsum_d = nc.declare_dram_parameter("rowsum", [128, RS_PARTS * Q], F32, isOutput=True)
    tgt_d = nc.declare_dram_parameter("tgt", [128, Q], F32, isOutput=True)
    mid_d = nc.declare_dram_parameter("mid", [1, 6 * 512], F32, isOutput=True)
    edge_d = nc.declare_dram_parameter("edge", [1, 14 * 128], F32, isOutput=True)

    with tile.TileContext(nc) as tc:
        with (
            tc.tile_pool(name="zp", bufs=1) as zp,
            tc.tile_pool(name="const", bufs=1) as constp,
            tc.tile_pool(name="expp", bufs=8) as expp,
            tc.tile_pool(name="dgp", bufs=2) as dgp,
            tc.tile_pool(name="stats", bufs=1) as statsp,
            tc.tile_pool(name="gram", bufs=2, space="PSUM") as gramp,
            tc.tile_pool(name="cacc", bufs=1, space="PSUM") as caccp,
        ):
            # ACT exp-table preload on a zeroed tile, overlapping the z DMA
            zeros = statsp.tile([32, 512], F32)
            nc.any.memset(zeros[:], 0.0)
            warm = statsp.tile([32, 2], F32)
            nc.scalar.activation(
                warm[:, 0:1], zeros[:, 0:1], mybir.ActivationFunctionType.Exp
            )

            # Spread the input DMA issue across idle engine sequencers: a
            # single sequencer spends ~620ns per dma_start dispatch, so 7
            # serial dispatches on sync would delay the last z chunk by 4us.
            z = zp.tile([128, 2, ZCOLS], FP8)
            idt = constp.tile([128, 128], F32)
            ones = constp.tile([128, 32], BF16)
            issuers = [nc.sync, nc.scalar, nc.gpsimd, nc.sync, nc.scalar]
            for (c0, c1), eng in zip(ZCHUNKS, issuers):
                eng.dma_start(z[:, :, c0:c1], zdr[:, :, c0:c1])
            nc.gpsimd.dma_start(idt[:], idt_d[:])
            nc.sync.dma_start(ones[:], ones_d[:])

            rowsum = statsp.tile([128, RS_PARTS * Q], F32)
            tgtt = statsp.tile([128, Q], F32)
            edgestage = statsp.tile([32, 14 * 128], F32)
            midstage = statsp.tile([32, 6 * 512], F32)
            # long-lived col-sum accumulator banks (chunks cc=2..5; banks
            # reused at the tail for cc=6,7 and the t>=32 edge tiles)
            macc = [
                caccp.tile([128, 512], F32, tag=f"M{i}", name=f"macc{i}")
                for i in range(4)
            ]

            def gram_mm(out_ap, qs, c0, c1):
                """out = z[:, qs:qs+128].T @ z[:, c0:c1] (scaled x256)."""
                nc.tensor.matmul(
                    out_ap,
                    lhsT=z[:, :, qs:qs + 128],
                    rhs=z[:, :, c0:c1],
                    start=True,
                    stop=True,
                    perf_mode=mybir.MatmulPerfMode.DoubleRow,
                )

            def dve_copy(dst, src):
                # PSUM -> SBUF drain; only one non-scalar input may be PSUM
                w = src.shape[-1]
                nc.vector.scalar_tensor_tensor(
                    out=dst,
                    in0=src,
                    scalar=0.0,
                    in1=zeros[:, 0:w],
                    op0=mybir.AluOpType.bypass,
                    op1=mybir.AluOpType.add,
                )

            ets = []

            # et offset of arc column x (x = rolled col - qs): et_off = x
            def colsum(granule, tq, x0, w, start, stop):
                nc.tensor.matmul(
                    granule,
                    lhsT=ones[:],
                    rhs=ets[tq][:, x0:x0 + w],
                    start=start,
                    stop=stop,
                )

            def mid_colsum(q2, cc):
                """Chunk cc (target tiles 4cc..4cc+3): PE accumulates across
                q2 in an exclusive PSUM bank."""
                colsum(
                    macc[cc - 2][0:32, 0:512], q2, 512 * cc - 128 * q2, 512,
                    start=(q2 == 0), stop=(q2 == Q - 1),
                )

            def emit_edge(t, e, granule):
                """All contributions for edge tile t, back-to-back (brief
                transient group), then drained to SBUF staging."""
                lo = max(0, t - 31)
                hi = min(7, t - 1)
                for q2 in range(lo, hi + 1):
                    colsum(
                        granule, q2, 128 * (t - q2), 128,
                        start=(q2 == lo), stop=(q2 == hi),
                    )
                dve_copy(edgestage[:, 128 * e:128 * (e + 1)], granule)

            for q in range(Q):
                qs = 128 * q
                et = expp.tile([128, COLS], BF16, tag="et")
                ets.append(et)
                # gram blocks B0..B3: 1024-wide slabs, 2x512 matmuls each,
                # exp'd as soon as written; col-sum filler interleaved so
                # the PE never idles waiting on ACT to free a PSUM slot
                pb = []
                for b in range(2):
                    p = gramp.tile([128, 1024], F32, tag="g")
                    base = qs + 1024 * b
                    gram_mm(p[:, 0:512], qs, base, base + 512)
                    gram_mm(p[:, 512:1024], qs, base + 512, base + 1024)
                    pb.append(p)
                # B0's rowsum partial rides the exp via the ACT accumulator
                nc.scalar.activation(
                    et[:, 0:1024], pb[0][:],
                    mybir.ActivationFunctionType.Exp, scale=EXP_SCALE,
                    accum_out=rowsum[:, RS_PARTS * q:RS_PARTS * q + 1],
                )
                if q >= 1:
                    mid_colsum(q - 1, 2)
                nc.scalar.activation(
                    et[:, 1024:2048], pb[1][:],
                    mybir.ActivationFunctionType.Exp, scale=EXP_SCALE,
                )
                nc.vector.tensor_reduce(
                    out=rowsum[:, RS_PARTS * q + 1:RS_PARTS * q + 2],
                    in_=et[:, 1024:2048],
                    axis=mybir.AxisListType.X,
                    op=mybir.AluOpType.add,
                )
                for b in range(2, 4):
                    p = gramp.tile([128, 1024], F32, tag="g")
                    base = qs + 1024 * b
                    gram_mm(p[:, 0:512], qs, base, base + 512)
                    gram_mm(p[:, 512:1024], qs, base + 512, base + 1024)
                    if q >= 1:
                        mid_colsum(q - 1, b + 1)
                    if b == 3:
                        nc.vector.tensor_reduce(
                            out=rowsum[:, RS_PARTS * q + 2:RS_PARTS * q + 3],
                            in_=et[:, 2048:3072],
                            axis=mybir.AxisListType.X,
                            op=mybir.AluOpType.add,
                        )
                    nc.scalar.activation(
                        et[:, 1024 * b:1024 * (b + 1)], p[:],
                        mybir.ActivationFunctionType.Exp, scale=EXP_SCALE,
                    )
                # C tile: B4 gram (the o=32 tile [qs+4096, qs+4224)) in its
                # first bank, edge-tile accumulation group in its second
                # bank -- keeps the per-q allocation count at 5 so next q's
                # B0 slot is gated by exp(B3), not by exp(B4)/dg.
                ct = gramp.tile([128, 1024], F32, tag="g")
                gram_mm(ct[:, 0:128], qs, qs + 4096, qs + 4224)
                if q >= 1:
                    mid_colsum(q - 1, 5)
                    # intra-core edge tile t=q needs ets[0..q-1] only
                    emit_edge(q, q - 1, ct[0:32, 512:640])
                dg = dgp.tile([128, 128], F32, tag="dg")
                nc.vector.tensor_mul(dg[:], ct[:, 0:128], idt[:])
                nc.vector.tensor_reduce(
                    out=tgtt[:, q:q + 1],
                    in_=dg[:],
                    axis=mybir.AxisListType.X,
                    op=mybir.AluOpType.add,
                )
                nc.scalar.activation(
                    et[:, 4096:4224], ct[:, 0:128],
                    mybir.ActivationFunctionType.Exp, scale=EXP_SCALE,
                )
                # B3+B4 are contiguous in et: one DVE partial for [3072:4224)
                nc.vector.tensor_reduce(
                    out=rowsum[:, RS_PARTS * q + 3:RS_PARTS * q + 4],
                    in_=et[:, 3072:4224],
                    axis=mybir.AxisListType.X,
                    op=mybir.AluOpType.add,
                )

            for cc in range(2, 6):
                mid_colsum(Q - 1, cc)

            # tail: cc=6,7 run through fresh gram-pool tiles (their banks
            # are free after the last exp) so they don't wait on the cc=2..5
            # drains; the 7 inter-core edge tiles round-robin over all 4
            # freed macc banks so the PE streams while the DVE drains trail
            for cc in range(2, 6):
                dve_copy(
                    midstage[:, 512 * (cc - 2):512 * (cc - 1)],
                    macc[cc - 2][0:32, 0:512],
                )
            for cc in (6, 7):
                gt = gramp.tile([128, 1024], F32, tag="g")
                gran = gt[0:32, 0:512]
                for q2 in range(Q):
                    colsum(
                        gran, q2, 512 * cc - 128 * q2, 512,
                        start=(q2 == 0), stop=(q2 == 7),
                    )
                dve_copy(midstage[:, 512 * (cc - 2):512 * (cc - 1)], gran)
            for e, t in enumerate(EDGE_TILES):
                if t < 32:
                    continue   # done mid-loop
                gran = macc[e % 4][0:32, 0:128]
                emit_edge(t, e, gran)

            # outputs fan out across sequencers so the 4 dispatches overlap
            nc.sync.dma_start(edge_d[:], edgestage[0:1, :])
            nc.scalar.dma_start(mid_d[:], midstage[0:1, :])
            nc.gpsimd.dma_start(rowsum_d[:], rowsum[:])
            nc.sync.dma_start(tgt_d[:], tgtt[:])

    nc.compile()
    return nc


def make_in_maps(z1: np.ndarray, z2: np.ndarray) -> list[dict]:
    z = np.concatenate([np.asarray(z1), np.asarray(z2)], axis=0).astype(np.float64)
    zn = z / np.maximum(np.linalg.norm(z, axis=1, keepdims=True), 1e-12)
    ztn = np.ascontiguousarray(zn.T * ZSCALE)  # [256, 8192]
    zdt = mybir.dt.np(FP8)
    ident = np.eye(128, dtype=np.float32)
    onesm = np.ones((128, 32), dtype=mybir.dt.np(BF16))
    in_maps = []
    for c in range(NCORES):
        rolled = np.roll(ztn, -1024 * c, axis=1)[:, :ZCOLS]
        # DoubleRow layout: [partition p, ko, x] = row (128*ko + p)
        zdr = np.ascontiguousarray(
            rolled.reshape(2, 128, ZCOLS).transpose(1, 0, 2)
        ).astype(zdt)
        in_maps.append({"zdr": zdr, "idt": ident, "ones": onesm})
    return in_maps


def assemble(results: list[dict]) -> np.float32:
    S = np.zeros(N, dtype=np.float64)
    tgt_all = np.zeros(N, dtype=np.float64)
    for c in range(NCORES):
        r = results[c]
        rowsum = (
            r["rowsum"].astype(np.float64).reshape(128, Q, RS_PARTS).sum(axis=-1)
        )
        tgt = r["tgt"].astype(np.float64)
        mid = r["mid"].astype(np.float64).reshape(6, 512)
        edge = r["edge"].astype(np.float64).reshape(14, 128)
        base = 1024 * c
        for q in range(Q):
            S[base + 128 * q: base + 128 * (q + 1)] += rowsum[:, q]
            tgt_all[base + 128 * q: base + 128 * (q + 1)] = tgt[:, q]
        for cc in range(2, 8):
            gidx = (512 * cc + np.arange(512) + base) % N
            S[gidx] += mid[cc - 2]
        for e, t in enumerate(EDGE_TILES):
            gidx = (128 * t + np.arange(128) + base) % N
            S[gidx] += edge[e]
    loss = np.mean(np.log(S) - tgt_all / (ZSCALE * ZSCALE * TEMP))
    return np.float32(loss)


_NC_CACHE: list = []


def kernel(z1: np.ndarray, z2: np.ndarray) -> np.ndarray:
    in_maps = make_in_maps(z1, z2)
    if not _NC_CACHE:
        _NC_CACHE.append(build_nc())
    nc = _NC_CACHE[0]
    res = run_bass_kernel_spmd(nc, in_maps, list(range(NCORES)))
    return assemble(res.results)


if __name__ == "__main__":
    rng = np.random.default_rng(0)
    z1 = rng.standard_normal((B, D), dtype=np.float32)
    z2 = rng.standard_normal((B, D), dtype=np.float32)
    print(kernel(z1, z2))


# revision 30
# speedup vs baseline: 1.3740x; 1.0248x over previous
"""NT-Xent contrastive loss (SimCLR-style) on 8 Trainium2 NeuronCores.

Problem: z1, z2 [4096, 256] fp32 -> scalar loss.
  zn = l2norm(z), z = concat(z1, z2) -> [8192, 256]
  sim = zn @ zn.T / 0.07              -> [8192, 8192]
  loss = -mean_i log_softmax(sim)[i, partner(i)],  partner(i) = (i + 4096) % 8192

Strategy (symmetric): exp(sim) is symmetric, so each unordered tile pair
{a, b} of the 64x64 grid of 128x128 blocks is computed ONCE. The core
owning row-tile a computes blocks (a, a+o mod 64) for o = 0..32 (the
o=32 pair is computed by both owners: 3% redundancy that keeps the
program SPMD-identical). Row sums come for free from the DVE via
tensor_scalar's accum_out (runs in the 4x perf mode on packed bf16, so
~1.1us per row tile instead of tensor_reduce's 4.5us); the transpose
credit for o = 1..31 comes from COLUMN sums of the same exp block,
computed on the PE as ones^T @ E matmuls accumulating in PSUM. Host
adds the per-core partial sums.

Per-core input is the normalized z^T rolled so its own 1024 rows sit at
columns [0, 1024): every core runs one identical program, and the o-arcs
become contiguous column ranges [0, 5120) -- only 62.5% of z is even
loaded. Matmuls run in fp8e4m3 (values pre-scaled x16) with DoubleRow
packing K=256 into one pass; exp runs on the scalar engine PSUM->SBUF in
bf16. Tolerance is rel 2e-2; fp8 error lands ~1e-3.

exp SBUF layout per q: [o0 | o1 | ... | o31 | o32] (4096 + 128 = 4224),
so gram blocks are clean 1024-wide slabs [qs+1024b, qs+1024(b+1)) that
align with the z DMA chunks, and the o=32 tail (which needs the last z
columns) is emitted LAST -- the first matmul only waits on the first
1024-column DMA chunk. Column-sum matmuls are interleaved between gram
blocks as PE filler so the tensor engine stays busy (and stays out of
the low p-states) while ACT drains the previous block.

PSUM discipline: an accumulation group conflicts with any other group
opened in the same bank while it is live, so long-lived accumulators
get exclusive banks. Banks 0-3: gram double-buffer ([128,1024] x 2).
Banks 4-7: col-sum chunks cc=2..5 (target tiles 8..23), held open
across the whole q loop. Chunks cc=6,7 and the 7 inter-core edge tiles
(t=32..38) run at the tail through the drained banks; the 7 intra-core
edge tiles (t=1..7) run mid-loop as brief transient groups inside a
gram slot (their DVE drain is quick now that the DVE is nearly idle).
"""

import numpy as np

import concourse.bass as bass
import concourse.tile as tile
from concourse import bacc, mybir
from concourse.bass_utils import run_bass_kernel_spmd

B, D = 4096, 256
N = 2 * B            # 8192 embeddings
NCORES = 8
NT = N // 128        # 64 tiles of 128 embeddings
Q = 8                # row tiles per core
ARC = 33             # column tiles per row tile (o = 0..32)
COLS = ARC * 128     # 4224
OCOLS = 32 * 128     # 4096: o0..o31 region; o32 tail at [4096, 4224)
ZCOLS = (Q - 1 + 32 + 1) * 128   # 5120 rolled columns needed per core
TEMP = 0.07
ZSCALE = 16.0        # pre-scale before fp8 cast (keeps values in e4m3 normal range)
EXP_SCALE = 1.0 / (ZSCALE * ZSCALE * TEMP)

F32 = mybir.dt.float32
BF16 = mybir.dt.bfloat16
FP8 = mybir.dt.float8e4

EDGE_TILES = list(range(1, 8)) + list(range(32, 39))

# Rowsum strategy: the DVE reduce-with-accumulator paths all run at
# 1 el/cycle on hw (the 2x/4x packed modes don't apply to reductions), so
# a monolithic 4224-wide reduce costs 4.5us and serializes the pipeline.
# Split it instead: the B0 block's sum rides the ACT exp instruction via
# accum_out (~0.2-0.3us accumulator read), and B1, B2, B3+B4 get their
# own DVE tensor_reduce partials (~1.1us each, interleaved with drains).
# Host adds the 4 partials per row tile.
RS_PARTS = 4         # rowsum partials per q: [ACT B0, DVE B1, DVE B2, DVE B3+B4]

# z DMA column chunks, issued in ascending order across three engine
# sequencers so early gram matmuls only wait on the small first chunk.
ZCHUNKS = [(0, 512), (512, 1536), (1536, 2560), (2560, 3584), (3584, 4608),
           (4608, 5120)]


def build_nc() -> bass.Bass:
    nc = bacc.Bacc("TRN2", target_bir_lowering=False, debug=False, num_devices=NCORES)
    zdr = nc.declare_dram_parameter("zdr", [128, 2, ZCOLS], FP8, isOutput=False)
    ones_d = nc.declare_dram_parameter("ones", [128, 32], BF16, isOutput=False)
    rowsum_d = nc.declare_dram_parameter("rowsum", [128, RS_PARTS * Q], F32, isOutput=True)
    o32_d = nc.declare_dram_parameter("o32", [128, Q * 128], F32, isOutput=True)
    mid_d = nc.declare_dram_parameter("mid", [1, 6 * 512], F32, isOutput=True)
    edge_d = nc.declare_dram_parameter("edge", [1, 14 * 128], F32, isOutput=True)

    with tile.TileContext(nc) as tc:
        with (
            tc.tile_pool(name="zp", bufs=1) as zp,
            tc.tile_pool(name="const", bufs=1) as constp,
            tc.tile_pool(name="expp", bufs=8) as expp,
            tc.tile_pool(name="stats", bufs=1) as statsp,
            tc.tile_pool(name="gram", bufs=2, space="PSUM") as gramp,
            tc.tile_pool(name="cacc", bufs=1, space="PSUM") as caccp,
        ):
            # ACT exp-table preload on a zeroed tile, overlapping the z DMA
            zeros = statsp.tile([128, 512], F32)
            nc.any.memset(zeros[:], 0.0)
            warm = statsp.tile([32, 2], F32)
            nc.scalar.activation(
                warm[:, 0:1], zeros[0:32, 0:1], mybir.ActivationFunctionType.Exp
            )

            # Spread the input DMA issue across idle engine sequencers: a
            # single sequencer spends ~620ns per dma_start dispatch, so 7
            # serial dispatches on sync would delay the last z chunk by 4us.
            z = zp.tile([128, 2, ZCOLS], FP8)
            ones = constp.tile([128, 32], BF16)
            issuers = [nc.sync, nc.scalar, nc.gpsimd, nc.sync, nc.scalar, nc.gpsimd]
            for (c0, c1), eng in zip(ZCHUNKS, issuers):
                eng.dma_start(z[:, :, c0:c1], zdr[:, :, c0:c1])
            nc.sync.dma_start(ones[:], ones_d[:])

            rowsum = statsp.tile([128, RS_PARTS * Q], F32)
            o32stage = statsp.tile([128, Q * 128], F32)
            edgestage = statsp.tile([32, 14 * 128], F32)
            midstage = statsp.tile([32, 6 * 512], F32)
            # long-lived col-sum accumulator banks (chunks cc=2..5; banks
            # reused at the tail for cc=6,7 and the t>=32 edge tiles)
            macc = [
                caccp.tile([128, 512], F32, tag=f"M{i}", name=f"macc{i}")
                for i in range(4)
            ]

            def gram_mm(out_ap, qs, c0, c1):
                """out = z[:, qs:qs+128].T @ z[:, c0:c1] (scaled x256)."""
                nc.tensor.matmul(
                    out_ap,
                    lhsT=z[:, :, qs:qs + 128],
                    rhs=z[:, :, c0:c1],
                    start=True,
                    stop=True,
                    perf_mode=mybir.MatmulPerfMode.DoubleRow,
                )

            def dve_copy(dst, src):
                # PSUM -> SBUF drain; only one non-scalar input may be PSUM
                p = src.shape[0]
                w = src.shape[-1]
                nc.vector.scalar_tensor_tensor(
                    out=dst,
                    in0=src,
                    scalar=0.0,
                    in1=zeros[0:p, 0:w],
                    op0=mybir.AluOpType.bypass,
                    op1=mybir.AluOpType.add,
                )

            ets = []

            # et offset of arc column x (x = rolled col - qs): et_off = x
            def colsum(granule, tq, x0, w, start, stop):
                nc.tensor.matmul(
                    granule,
                    lhsT=ones[:],
                    rhs=ets[tq][:, x0:x0 + w],
                    start=start,
                    stop=stop,
                )

            def mid_colsum(q2, cc):
                """Chunk cc (target tiles 4cc..4cc+3): PE accumulates across
                q2 in an exclusive PSUM bank."""
                colsum(
                    macc[cc - 2][0:32, 0:512], q2, 512 * cc - 128 * q2, 512,
                    start=(q2 == 0), stop=(q2 == Q - 1),
                )

            def emit_edge(t, e, granule):
                """All contributions for edge tile t, back-to-back (brief
                transient group), then drained to SBUF staging."""
                lo = max(0, t - 31)
                hi = min(7, t - 1)
                for q2 in range(lo, hi + 1):
                    colsum(
                        granule, q2, 128 * (t - q2), 128,
                        start=(q2 == lo), stop=(q2 == hi),
                    )
                dve_copy(edgestage[:, 128 * e:128 * (e + 1)], granule)

            for q in range(Q):
                qs = 128 * q
                et = expp.tile([128, OCOLS], BF16, tag="et")
                ets.append(et)
                # gram blocks B0..B3: 1024-wide slabs, 2x512 matmuls each,
                # exp'd as soon as written; col-sum filler interleaved so
                # the PE never idles waiting on ACT to free a PSUM slot
                pb = []
                for b in range(2):
                    p = gramp.tile([128, 1024], F32, tag="g")
                    base = qs + 1024 * b
                    gram_mm(p[:, 0:512], qs, base, base + 512)
                    gram_mm(p[:, 512:1024], qs, base + 512, base + 1024)
                    pb.append(p)
                # B0's rowsum partial rides the exp via the ACT accumulator
                nc.scalar.activation(
                    et[:, 0:1024], pb[0][:],
                    mybir.ActivationFunctionType.Exp, scale=EXP_SCALE,
                    accum_out=rowsum[:, RS_PARTS * q:RS_PARTS * q + 1],
                )
                if q >= 1:
                    mid_colsum(q - 1, 2)
                nc.scalar.activation(
                    et[:, 1024:2048], pb[1][:],
                    mybir.ActivationFunctionType.Exp, scale=EXP_SCALE,
                )
                nc.vector.tensor_reduce(
                    out=rowsum[:, RS_PARTS * q + 1:RS_PARTS * q + 2],
                    in_=et[:, 1024:2048],
                    axis=mybir.AxisListType.X,
                    op=mybir.AluOpType.add,
                )
                for b in range(2, 4):
                    p = gramp.tile([128, 1024], F32, tag="g")
                    base = qs + 1024 * b
                    gram_mm(p[:, 0:512], qs, base, base + 512)
                    gram_mm(p[:, 512:1024], qs, base + 512, base + 1024)
                    if q >= 1:
                        mid_colsum(q - 1, b + 1)
                    if b == 3:
                        nc.vector.tensor_reduce(
                            out=rowsum[:, RS_PARTS * q + 2:RS_PARTS * q + 3],
                            in_=et[:, 2048:3072],
                            axis=mybir.AxisListType.X,
                            op=mybir.AluOpType.add,
                        )
                    nc.scalar.activation(
                        et[:, 1024 * b:1024 * (b + 1)], p[:],
                        mybir.ActivationFunctionType.Exp, scale=EXP_SCALE,
                    )
                # C tile: the raw o=32 gram block [qs+4096, qs+4224) in its
                # first bank (drained to SBUF and shipped to the host, which
                # computes both its exp row-sum contribution and the
                # target-pair diagonal in fp64 -- no exp/diag work on
                # device), edge-tile accumulation group in its second bank.
                ct = gramp.tile([128, 1024], F32, tag="g")
                gram_mm(ct[:, 0:128], qs, qs + 4096, qs + 4224)
                if q >= 1:
                    mid_colsum(q - 1, 5)
                dve_copy(o32stage[:, 128 * q:128 * (q + 1)], ct[:, 0:128])
                if q >= 1:
                    # intra-core edge tile t=q needs ets[0..q-1] only
                    emit_edge(q, q - 1, ct[0:32, 512:640])
                nc.vector.tensor_reduce(
                    out=rowsum[:, RS_PARTS * q + 3:RS_PARTS * q + 4],
                    in_=et[:, 3072:4096],
                    axis=mybir.AxisListType.X,
                    op=mybir.AluOpType.add,
                )

            for cc in range(2, 6):
                mid_colsum(Q - 1, cc)

            # tail: cc=6,7 run through fresh gram-pool tiles (their banks
            # are free after the last exp) so they don't wait on the cc=2..5
            # drains; the 7 inter-core edge tiles round-robin over all 4
            # freed macc banks so the PE streams while the DVE drains trail
            for cc in range(2, 6):
                dve_copy(
                    midstage[:, 512 * (cc - 2):512 * (cc - 1)],
                    macc[cc - 2][0:32, 0:512],
                )
            for cc in (6, 7):
                gt = gramp.tile([128, 1024], F32, tag="g")
                gran = gt[0:32, 0:512]
                for q2 in range(Q):
                    colsum(
                        gran, q2, 512 * cc - 128 * q2, 512,
                        start=(q2 == 0), stop=(q2 == 7),
                    )
                dve_copy(midstage[:, 512 * (cc - 2):512 * (cc - 1)], gran)
            for e, t in enumerate(EDGE_TILES):
                if t < 32:
                    continue   # done mid-loop
                gran = macc[e % 4][0:32, 0:128]
                emit_edge(t, e, gran)

            # outputs fan out across sequencers so the 4 dispatches overlap
            nc.sync.dma_start(edge_d[:], edgestage[0:1, :])
            nc.scalar.dma_start(mid_d[:], midstage[0:1, :])
            nc.gpsimd.dma_start(rowsum_d[:], rowsum[:])
            nc.sync.dma_start(o32_d[:], o32stage[:])

    nc.compile()
    return nc


def make_in_maps(z1: np.ndarray, z2: np.ndarray) -> list[dict]:
    z = np.concatenate([np.asarray(z1), np.asarray(z2)], axis=0).astype(np.float64)
    zn = z / np.maximum(np.linalg.norm(z, axis=1, keepdims=True), 1e-12)
    ztn = np.ascontiguousarray(zn.T * ZSCALE)  # [256, 8192]
    zdt = mybir.dt.np(FP8)
    onesm = np.ones((128, 32), dtype=mybir.dt.np(BF16))
    in_maps = []
    for c in range(NCORES):
        rolled = np.roll(ztn, -1024 * c, axis=1)[:, :ZCOLS]
        # DoubleRow layout: [partition p, ko, x] = row (128*ko + p)
        zdr = np.ascontiguousarray(
            rolled.reshape(2, 128, ZCOLS).transpose(1, 0, 2)
        ).astype(zdt)
        in_maps.append({"zdr": zdr, "ones": onesm})
    return in_maps


def assemble(results: list[dict]) -> np.float32:
    S = np.zeros(N, dtype=np.float64)
    tgt_all = np.zeros(N, dtype=np.float64)
    for c in range(NCORES):
        r = results[c]
        rowsum = (
            r["rowsum"].astype(np.float64).reshape(128, Q, RS_PARTS).sum(axis=-1)
        )
        # raw o=32 gram blocks [p, q*128+j]: exp row-sums and the
        # target-pair diagonal both come out in fp64 here
        o32 = r["o32"].astype(np.float64).reshape(128, Q, 128)
        rowsum += np.exp(o32 * EXP_SCALE).sum(axis=-1)
        tgt = o32[np.arange(128), :, np.arange(128)]  # [128, Q]
        mid = r["mid"].astype(np.float64).reshape(6, 512)
        edge = r["edge"].astype(np.float64).reshape(14, 128)
        base = 1024 * c
        for q in range(Q):
            S[base + 128 * q: base + 128 * (q + 1)] += rowsum[:, q]
            tgt_all[base + 128 * q: base + 128 * (q + 1)] = tgt[:, q]
        for cc in range(2, 8):
            gidx = (512 * cc + np.arange(512) + base) % N
            S[gidx] += mid[cc - 2]
        for e, t in enumerate(EDGE_TILES):
            gidx = (128 * t + np.arange(128) + base) % N
            S[gidx] += edge[e]
    loss = np.mean(np.log(S) - tgt_all / (ZSCALE * ZSCALE * TEMP))
    return np.float32(loss)


_NC_CACHE: list = []


def kernel(z1: np.ndarray, z2: np.ndarray) -> np.ndarray:
    in_maps = make_in_maps(z1, z2)
    if not _NC_CACHE:
        _NC_CACHE.append(build_nc())
    nc = _NC_CACHE[0]
    res = run_bass_kernel_spmd(nc, in_maps, list(range(NCORES)))
    return assemble(res.results)


if __name__ == "__main__":
    rng = np.random.default_rng(0)
    z1 = rng.standard_normal((B, D), dtype=np.float32)
    z2 = rng.standard_normal((B, D), dtype=np.float32)
    print(kernel(z1, z2))


# revision 31
# speedup vs baseline: 1.3917x; 1.0129x over previous
"""NT-Xent contrastive loss (SimCLR-style) on 8 Trainium2 NeuronCores.

Problem: z1, z2 [4096, 256] fp32 -> scalar loss.
  zn = l2norm(z), z = concat(z1, z2) -> [8192, 256]
  sim = zn @ zn.T / 0.07              -> [8192, 8192]
  loss = -mean_i log_softmax(sim)[i, partner(i)],  partner(i) = (i + 4096) % 8192

Strategy (symmetric): exp(sim) is symmetric, so each unordered tile pair
{a, b} of the 64x64 grid of 128x128 blocks is computed ONCE. The core
owning row-tile a computes blocks (a, a+o mod 64) for o = 0..32 (the
o=32 pair is computed by both owners: 3% redundancy that keeps the
program SPMD-identical). Row sums come for free from the DVE via
tensor_scalar's accum_out (runs in the 4x perf mode on packed bf16, so
~1.1us per row tile instead of tensor_reduce's 4.5us); the transpose
credit for o = 1..31 comes from COLUMN sums of the same exp block,
computed on the PE as ones^T @ E matmuls accumulating in PSUM. Host
adds the per-core partial sums.

Per-core input is the normalized z^T rolled so its own 1024 rows sit at
columns [0, 1024): every core runs one identical program, and the o-arcs
become contiguous column ranges [0, 5120) -- only 62.5% of z is even
loaded. Matmuls run in fp8e4m3 (values pre-scaled x16) with DoubleRow
packing K=256 into one pass; exp runs on the scalar engine PSUM->SBUF in
bf16. Tolerance is rel 2e-2; fp8 error lands ~1e-3.

exp SBUF layout per q: [o0 | o1 | ... | o31 | o32] (4096 + 128 = 4224),
so gram blocks are clean 1024-wide slabs [qs+1024b, qs+1024(b+1)) that
align with the z DMA chunks, and the o=32 tail (which needs the last z
columns) is emitted LAST -- the first matmul only waits on the first
1024-column DMA chunk. Column-sum matmuls are interleaved between gram
blocks as PE filler so the tensor engine stays busy (and stays out of
the low p-states) while ACT drains the previous block.

PSUM discipline: an accumulation group conflicts with any other group
opened in the same bank while it is live, so long-lived accumulators
get exclusive banks. Banks 0-3: gram double-buffer ([128,1024] x 2).
Banks 4-7: col-sum chunks cc=2..5 (target tiles 8..23), held open
across the whole q loop. Chunks cc=6,7 and the 7 inter-core edge tiles
(t=32..38) run at the tail through the drained banks; the 7 intra-core
edge tiles (t=1..7) run mid-loop as brief transient groups inside a
gram slot (their DVE drain is quick now that the DVE is nearly idle).
"""

import numpy as np

import concourse.bass as bass
import concourse.tile as tile
from concourse import bacc, mybir
from concourse.bass_utils import run_bass_kernel_spmd

B, D = 4096, 256
N = 2 * B            # 8192 embeddings
NCORES = 8
NT = N // 128        # 64 tiles of 128 embeddings
Q = 8                # row tiles per core
ARC = 33             # column tiles per row tile (o = 0..32)
COLS = ARC * 128     # 4224
OCOLS = 32 * 128     # 4096: o0..o31 region; o32 tail at [4096, 4224)
ZCOLS = (Q - 1 + 32 + 1) * 128   # 5120 rolled columns needed per core
TEMP = 0.07
ZSCALE = 16.0        # pre-scale before fp8 cast (keeps values in e4m3 normal range)
EXP_SCALE = 1.0 / (ZSCALE * ZSCALE * TEMP)

F32 = mybir.dt.float32
BF16 = mybir.dt.bfloat16
FP8 = mybir.dt.float8e4

EDGE_TILES = list(range(1, 8)) + list(range(32, 39))

# Rowsum strategy: the DVE reduce-with-accumulator paths all run at
# 1 el/cycle on hw (the 2x/4x packed modes don't apply to reductions), so
# a monolithic 4224-wide reduce costs 4.5us and serializes the pipeline.
# Split it instead: the B0 block's sum rides the ACT exp instruction via
# accum_out (~0.2-0.3us accumulator read), and B1, B2, B3+B4 get their
# own DVE tensor_reduce partials (~1.1us each, interleaved with drains).
# Host adds the 4 partials per row tile.
RS_PARTS = 4         # rowsum partials per q: [ACT B0, DVE B1, DVE B2, DVE B3+B4]

# z DMA column chunks, issued in ascending order across three engine
# sequencers so early gram matmuls only wait on the small first chunk.
ZCHUNKS = [(0, 512), (512, 1536), (1536, 2560), (2560, 3584), (3584, 4608),
           (4608, 5120)]


def build_nc() -> bass.Bass:
    nc = bacc.Bacc("TRN2", target_bir_lowering=False, debug=False, num_devices=NCORES)
    zdr = nc.declare_dram_parameter("zdr", [128, 2, ZCOLS], FP8, isOutput=False)
    ones_d = nc.declare_dram_parameter("ones", [128, 32], BF16, isOutput=False)
    rowsum_d = nc.declare_dram_parameter("rowsum", [128, RS_PARTS * Q], F32, isOutput=True)
    o32_d = nc.declare_dram_parameter("o32", [128, Q * 128], F32, isOutput=True)
    mid_d = nc.declare_dram_parameter("mid", [1, 6 * 512], F32, isOutput=True)
    edge_d = nc.declare_dram_parameter("edge", [1, 14 * 128], F32, isOutput=True)

    with tile.TileContext(nc) as tc:
        with (
            tc.tile_pool(name="zp", bufs=1) as zp,
            tc.tile_pool(name="const", bufs=1) as constp,
            tc.tile_pool(name="expp", bufs=8) as expp,
            tc.tile_pool(name="stats", bufs=1) as statsp,
            tc.tile_pool(name="gram", bufs=2, space="PSUM") as gramp,
            tc.tile_pool(name="cacc", bufs=1, space="PSUM") as caccp,
        ):
            # ACT exp-table preload on a zeroed tile, overlapping the z DMA
            zeros = statsp.tile([128, 512], F32)
            nc.any.memset(zeros[:], 0.0)
            warm = statsp.tile([32, 2], F32)
            nc.scalar.activation(
                warm[:, 0:1], zeros[0:32, 0:1], mybir.ActivationFunctionType.Exp
            )

            # Spread the input DMA issue across idle engine sequencers: a
            # single sequencer spends ~620ns per dma_start dispatch, so 7
            # serial dispatches on sync would delay the last z chunk by 4us.
            z = zp.tile([128, 2, ZCOLS], FP8)
            ones = constp.tile([128, 32], BF16)
            issuers = [nc.sync, nc.scalar, nc.gpsimd, nc.sync, nc.scalar, nc.gpsimd]
            for (c0, c1), eng in zip(ZCHUNKS, issuers):
                eng.dma_start(z[:, :, c0:c1], zdr[:, :, c0:c1])
            nc.sync.dma_start(ones[:], ones_d[:])

            rowsum = statsp.tile([128, RS_PARTS * Q], F32)
            o32stage = statsp.tile([128, Q * 128], F32)
            edgestage = statsp.tile([32, 14 * 128], F32)
            midstage = statsp.tile([32, 6 * 512], F32)
            # long-lived col-sum accumulator banks (chunks cc=2..5; banks
            # reused at the tail for cc=6,7 and the t>=32 edge tiles)
            macc = [
                caccp.tile([128, 512], F32, tag=f"M{i}", name=f"macc{i}")
                for i in range(4)
            ]

            def gram_mm(out_ap, qs, c0, c1):
                """out = z[:, qs:qs+128].T @ z[:, c0:c1] (scaled x256)."""
                nc.tensor.matmul(
                    out_ap,
                    lhsT=z[:, :, qs:qs + 128],
                    rhs=z[:, :, c0:c1],
                    start=True,
                    stop=True,
                    perf_mode=mybir.MatmulPerfMode.DoubleRow,
                )

            def dve_copy(dst, src):
                # PSUM -> SBUF drain; only one non-scalar input may be PSUM
                p = src.shape[0]
                w = src.shape[-1]
                nc.vector.scalar_tensor_tensor(
                    out=dst,
                    in0=src,
                    scalar=0.0,
                    in1=zeros[0:p, 0:w],
                    op0=mybir.AluOpType.bypass,
                    op1=mybir.AluOpType.add,
                )

            ets = []

            # et offset of arc column x (x = rolled col - qs): et_off = x
            def colsum(granule, tq, x0, w, start, stop):
                nc.tensor.matmul(
                    granule,
                    lhsT=ones[:],
                    rhs=ets[tq][:, x0:x0 + w],
                    start=start,
                    stop=stop,
                )

            def mid_colsum(q2, cc):
                """Chunk cc (target tiles 4cc..4cc+3): PE accumulates across
                q2 in an exclusive PSUM bank."""
                colsum(
                    macc[cc - 2][0:32, 0:512], q2, 512 * cc - 128 * q2, 512,
                    start=(q2 == 0), stop=(q2 == Q - 1),
                )

            def emit_edge(t, e, granule):
                """All contributions for edge tile t, back-to-back (brief
                transient group), then drained to SBUF staging."""
                lo = max(0, t - 31)
                hi = min(7, t - 1)
                for q2 in range(lo, hi + 1):
                    colsum(
                        granule, q2, 128 * (t - q2), 128,
                        start=(q2 == lo), stop=(q2 == hi),
                    )
                dve_copy(edgestage[:, 128 * e:128 * (e + 1)], granule)

            for q in range(Q):
                qs = 128 * q
                et = expp.tile([128, OCOLS], BF16, tag="et")
                ets.append(et)
                # gram blocks B0..B3: 1024-wide slabs, 2x512 matmuls each,
                # exp'd as soon as written; col-sum filler interleaved so
                # the PE never idles waiting on ACT to free a PSUM slot
                # col-sum fillers go BEFORE each gram pair on the PE queue:
                # ready work must not sit behind a gram that's waiting on an
                # exp to free its PSUM slot (in-order head-of-line), and a
                # busy PE stays out of the slow p-states.
                pb = []
                for b in range(2):
                    if q >= 1:
                        mid_colsum(q - 1, 2 + b)
                    p = gramp.tile([128, 1024], F32, tag="g")
                    base = qs + 1024 * b
                    gram_mm(p[:, 0:512], qs, base, base + 512)
                    gram_mm(p[:, 512:1024], qs, base + 512, base + 1024)
                    pb.append(p)
                # B0's rowsum partial rides the exp via the ACT accumulator
                nc.scalar.activation(
                    et[:, 0:1024], pb[0][:],
                    mybir.ActivationFunctionType.Exp, scale=EXP_SCALE,
                    accum_out=rowsum[:, RS_PARTS * q:RS_PARTS * q + 1],
                )
                nc.scalar.activation(
                    et[:, 1024:2048], pb[1][:],
                    mybir.ActivationFunctionType.Exp, scale=EXP_SCALE,
                )
                nc.vector.tensor_reduce(
                    out=rowsum[:, RS_PARTS * q + 1:RS_PARTS * q + 2],
                    in_=et[:, 1024:2048],
                    axis=mybir.AxisListType.X,
                    op=mybir.AluOpType.add,
                )
                for b in range(2, 4):
                    if q >= 1:
                        mid_colsum(q - 1, 2 + b)
                    p = gramp.tile([128, 1024], F32, tag="g")
                    base = qs + 1024 * b
                    gram_mm(p[:, 0:512], qs, base, base + 512)
                    gram_mm(p[:, 512:1024], qs, base + 512, base + 1024)
                    nc.scalar.activation(
                        et[:, 1024 * b:1024 * (b + 1)], p[:],
                        mybir.ActivationFunctionType.Exp, scale=EXP_SCALE,
                    )
                # C tile: the raw o=32 gram block [qs+4096, qs+4224) in its
                # first bank (drained to SBUF and shipped to the host, which
                # computes both its exp row-sum contribution and the
                # target-pair diagonal in fp64 -- no exp/diag work on
                # device), edge-tile accumulation group in its second bank.
                # Its readers are all quick DVE copies emitted BEFORE the big
                # reduces, so the slot recycles early for next q's B1 grams.
                ct = gramp.tile([128, 1024], F32, tag="g")
                gram_mm(ct[:, 0:128], qs, qs + 4096, qs + 4224)
                dve_copy(o32stage[:, 128 * q:128 * (q + 1)], ct[:, 0:128])
                if q >= 1:
                    # intra-core edge tile t=q needs ets[0..q-1] only
                    emit_edge(q, q - 1, ct[0:32, 512:640])
                nc.vector.tensor_reduce(
                    out=rowsum[:, RS_PARTS * q + 2:RS_PARTS * q + 3],
                    in_=et[:, 2048:3072],
                    axis=mybir.AxisListType.X,
                    op=mybir.AluOpType.add,
                )
                nc.vector.tensor_reduce(
                    out=rowsum[:, RS_PARTS * q + 3:RS_PARTS * q + 4],
                    in_=et[:, 3072:4096],
                    axis=mybir.AxisListType.X,
                    op=mybir.AluOpType.add,
                )

            for cc in range(2, 6):
                mid_colsum(Q - 1, cc)

            # tail: cc=6,7 run through fresh gram-pool tiles (their banks
            # are free after the last exp) so they don't wait on the cc=2..5
            # drains; the 7 inter-core edge tiles round-robin over all 4
            # freed macc banks so the PE streams while the DVE drains trail
            for cc in range(2, 6):
                dve_copy(
                    midstage[:, 512 * (cc - 2):512 * (cc - 1)],
                    macc[cc - 2][0:32, 0:512],
                )
            for cc in (6, 7):
                gt = gramp.tile([128, 1024], F32, tag="g")
                gran = gt[0:32, 0:512]
                for q2 in range(Q):
                    colsum(
                        gran, q2, 512 * cc - 128 * q2, 512,
                        start=(q2 == 0), stop=(q2 == 7),
                    )
                dve_copy(midstage[:, 512 * (cc - 2):512 * (cc - 1)], gran)
            for e, t in enumerate(EDGE_TILES):
                if t < 32:
                    continue   # done mid-loop
                gran = macc[e % 4][0:32, 0:128]
                emit_edge(t, e, gran)

            # outputs fan out across sequencers so the 4 dispatches overlap
            nc.sync.dma_start(edge_d[:], edgestage[0:1, :])
            nc.scalar.dma_start(mid_d[:], midstage[0:1, :])
            nc.gpsimd.dma_start(rowsum_d[:], rowsum[:])
            nc.sync.dma_start(o32_d[:], o32stage[:])

    nc.compile()
    return nc


def make_in_maps(z1: np.ndarray, z2: np.ndarray) -> list[dict]:
    z = np.concatenate([np.asarray(z1), np.asarray(z2)], axis=0).astype(np.float64)
    zn = z / np.maximum(np.linalg.norm(z, axis=1, keepdims=True), 1e-12)
    ztn = np.ascontiguousarray(zn.T * ZSCALE)  # [256, 8192]
    zdt = mybir.dt.np(FP8)
    onesm = np.ones((128, 32), dtype=mybir.dt.np(BF16))
    in_maps = []
    for c in range(NCORES):
        rolled = np.roll(ztn, -1024 * c, axis=1)[:, :ZCOLS]
        # DoubleRow layout: [partition p, ko, x] = row (128*ko + p)
        zdr = np.ascontiguousarray(
            rolled.reshape(2, 128, ZCOLS).transpose(1, 0, 2)
        ).astype(zdt)
        in_maps.append({"zdr": zdr, "ones": onesm})
    return in_maps


def assemble(results: list[dict]) -> np.float32:
    S = np.zeros(N, dtype=np.float64)
    tgt_all = np.zeros(N, dtype=np.float64)
    for c in range(NCORES):
        r = results[c]
        rowsum = (
            r["rowsum"].astype(np.float64).reshape(128, Q, RS_PARTS).sum(axis=-1)
        )
        # raw o=32 gram blocks [p, q*128+j]: exp row-sums and the
        # target-pair diagonal both come out in fp64 here
        o32 = r["o32"].astype(np.float64).reshape(128, Q, 128)
        rowsum += np.exp(o32 * EXP_SCALE).sum(axis=-1)
        tgt = o32[np.arange(128), :, np.arange(128)]  # [128, Q]
        mid = r["mid"].astype(np.float64).reshape(6, 512)
        edge = r["edge"].astype(np.float64).reshape(14, 128)
        base = 1024 * c
        for q in range(Q):
            S[base + 128 * q: base + 128 * (q + 1)] += rowsum[:, q]
            tgt_all[base + 128 * q: base + 128 * (q + 1)] = tgt[:, q]
        for cc in range(2, 8):
            gidx = (512 * cc + np.arange(512) + base) % N
            S[gidx] += mid[cc - 2]
        for e, t in enumerate(EDGE_TILES):
            gidx = (128 * t + np.arange(128) + base) % N
            S[gidx] += edge[e]
    loss = np.mean(np.log(S) - tgt_all / (ZSCALE * ZSCALE * TEMP))
    return np.float32(loss)


_NC_CACHE: list = []


def kernel(z1: np.ndarray, z2: np.ndarray) -> np.ndarray:
    in_maps = make_in_maps(z1, z2)
    if not _NC_CACHE:
        _NC_CACHE.append(build_nc())
    nc = _NC_CACHE[0]
    res = run_bass_kernel_spmd(nc, in_maps, list(range(NCORES)))
    return assemble(res.results)


if __name__ == "__main__":
    rng = np.random.default_rng(0)
    z1 = rng.standard_normal((B, D), dtype=np.float32)
    z2 = rng.standard_normal((B, D), dtype=np.float32)
    print(kernel(z1, z2))
